# revision 10
# baseline (speedup 1.0000x reference)
"""Trainium2 Bass kernel for nn_EquivariantProductBasisBlock (MACE product-basis block).

Self-contained: host-side sharding/preprocessing + Bass/Tile device kernel on 8 cores.

Math (validated vs reference): per node n, channel c, species s, x = x[n,c,:] in R^9:
    out[z] = sum_i x_i * F[z,i],   F = C1 + C2 @ x + C3h @ y,   y = {x_j x_k}_{j<=k}
with C* the species/channel coefficient tables folded from (u*, w*) on the host.
Then gate = f0 @ gate_kernel[s] + gate_bias[s]; f0*=gate[:C]; f1*=gate[C:];
out = [f0 @ lin0, f1 @ lin1] / sqrt(C).

Device layout: channels on SBUF partitions, nodes species-sorted so every tile is
species-pure (tile t = species t, 10 tiles/core). The runtime bills ~40-50us per
instruction regardless of engine or size, so the program minimizes instruction
count: one bulk input DMA, monomial build batched over tile pairs (3D APs),
the per-tile V@W contraction in 8 broadcast-mul + 8 reduce ops (SBUF-bound
granularity), fused bias+gating via scalar_tensor_tensor, and per-pair output DMA.

Species overflow beyond 1024 nodes/species is computed on the host in numpy.
"""

import numpy as np

N_CORES = 8
C, D, S = 128, 9, 10
NM = 45           # deg-2 monomials
NROW = 56         # host slots: 45 y | 9 x | 1 one | 1 pad (overflow path)
NM55 = 55         # device slots per channel: 45 y | 9 x | 1 one
TILE_N = 128
TPC = S           # tiles per core (one per species)
NODES_PER_CORE = TPC * TILE_N          # 1280
CAP_PER_SPECIES = N_CORES * TILE_N     # 1024 device-handled nodes per species

# monomials ordered by diagonal offset o=k-j then j: slot(o,j) = OSTART[o]+j.
# Each V-build op is then pure step-1 (out/in0/in1 all contiguous runs).
OSTART = [0] * D
for o in range(1, D):
    OSTART[o] = OSTART[o - 1] + (D - (o - 1))
MONO_JK = [(j, j + o) for o in range(D) for j in range(D - o)]


# ----------------------------------------------------------------------------
# host math
# ----------------------------------------------------------------------------

def _build_xr(node_feats):
    n = node_feats.shape[0]
    x = np.empty((n, C, D), np.float32)
    x[:, :, 0] = node_feats[:, :C]
    x[:, :, 1:4] = node_feats[:, C:4 * C].reshape(n, C, 3)
    x[:, :, 4:9] = node_feats[:, 4 * C:].reshape(n, C, 5)
    return x


def _build_coeff_tables(i):
    def c3h(u3, w3):
        c3 = np.einsum('zijkp,spc->sczijk', u3, w3[:, :, :], optimize=True)
        out = np.zeros(c3.shape[:4] + (NM,), np.float64)
        for m, (j, k) in enumerate(MONO_JK):
            out[..., m] = c3[..., j, k] if j == k else c3[..., j, k] + c3[..., k, j]
        return out

    def c2(u2, w2):
        return np.einsum('zijp,spc->sczij', u2, w2, optimize=True)

    def c1(u1, w1):
        return np.einsum('zip,spc->sczi', u1, w1, optimize=True)

    h0 = c3h(i['u3_0e'], i['w3_0e']); h1 = c3h(i['u3_1o'], i['w3_1o'])
    q0 = c2(i['u2_0e'], i['w2_0e']);  q1 = c2(i['u2_1o'], i['w2_1o'])
    l0 = c1(i['u1_0e'], i['w1_0e']);  l1 = c1(i['u1_1o'], i['w1_1o'])

    W = np.zeros((S, C, NROW, 36), np.float64)
    W[:, :, 0:45, 0:9] = np.moveaxis(h0[:, :, 0], -1, -2)
    W[:, :, 45:54, 0:9] = np.moveaxis(q0[:, :, 0], -1, -2)
    W[:, :, 54, 0:9] = l0[:, :, 0]
    for z in range(3):
        sl = slice(9 + z * 9, 18 + z * 9)
        W[:, :, 0:45, sl] = np.moveaxis(h1[:, :, z], -1, -2)
        W[:, :, 45:54, sl] = np.moveaxis(q1[:, :, z], -1, -2)
        W[:, :, 54, sl] = l1[:, :, z]
    return W.astype(np.float32)   # [S, C, 56, 36]


def _numpy_forward(inputs, idx):
    """Reference-equivalent host computation for node subset idx (overflow path)."""
    i = {k: np.asarray(v) for k, v in inputs.items()}
    nf = i['node_feats'][idx]; sp = i['node_species'][idx]
    xr = _build_xr(nf)
    W = _build_coeff_tables(i)
    n = nf.shape[0]
    V = np.empty((n, C, NROW), np.float32)
    for m, (j, k) in enumerate(MONO_JK):
        V[:, :, m] = xr[:, :, j] * xr[:, :, k]
    V[:, :, 45:54] = xr
    V[:, :, 54] = 1.0
    V[:, :, 55] = 0.0
    F = np.einsum('ncm,ncmz->ncz', V, W[sp], optimize=True)
    f = np.einsum('nczi,nci->ncz', F.reshape(n, C, 4, D), xr, optimize=True)
    f0, f1 = f[:, :, 0], f[:, :, 1:4]
    gate = np.einsum('nc,nck->nk', f0, i['gate_kernel'][sp], optimize=True) + i['gate_bias'][sp]
    f0g = f0 * gate[:, :C]
    f1g = f1 * gate[:, C:, None]
    inv = 1.0 / np.sqrt(np.float32(C))
    o0 = np.einsum('nc,ck->nk', f0g, i['lin_w_0e'], optimize=True) * inv
    o1 = np.einsum('ncd,ck->nkd', f1g, i['lin_w_1o'], optimize=True) * inv
    return np.concatenate([o0.reshape(n, C), o1.reshape(n, C * 3)], axis=1).astype(np.float32)


def _bf16(x):
    import ml_dtypes
    return np.asarray(x, np.float32).astype(ml_dtypes.bfloat16)


def host_prepare(inputs):
    """Returns (per_core_inmaps, device_rows [N_CORES,1280] global node ids (-1 pad),
    overflow_idx)."""
    i = {k: np.asarray(v) for k, v in inputs.items()}
    sp = i['node_species']

    order = np.argsort(sp, kind='stable')
    sorted_sp = sp[order]
    device_rows = np.full((N_CORES, NODES_PER_CORE), -1, np.int64)
    overflow = []
    for s in range(S):
        ids = order[sorted_sp == s]
        dev = ids[:CAP_PER_SPECIES]
        overflow.append(ids[CAP_PER_SPECIES:])
        for k in range(N_CORES):
            chunk = dev[k * TILE_N:(k + 1) * TILE_N]
            device_rows[k, s * TILE_N: s * TILE_N + len(chunk)] = chunk
    overflow_idx = np.concatenate(overflow) if overflow else np.zeros(0, np.int64)

    xr = _build_xr(i['node_feats'])                       # [N, C, 9]
    W = _build_coeff_tables(i)                            # [S, C, 56, 36]
    # channel-major coefficient table: wc[c, (s, m55, zi36)]
    wc_bf = _bf16(np.ascontiguousarray(
        W[:, :, :55, :].transpose(1, 0, 2, 3).reshape(C, S * 55 * 36)))

    gk = np.zeros((C, S * 2 * C), np.float32)             # rows c, col s*256 + j
    for s in range(S):
        gk[:, s * 256:(s + 1) * 256] = i['gate_kernel'][s]

    bias = np.zeros((C, S * 2), np.float32)               # rows k2%128, col s*2 + half
    for s in range(S):
        bias[:, 2 * s] = i['gate_bias'][s, :C]
        bias[:, 2 * s + 1] = i['gate_bias'][s, C:]

    inv = 1.0 / np.sqrt(np.float32(C))
    lin = np.concatenate([i['lin_w_0e'] * inv, i['lin_w_1o'] * inv], axis=1)  # [128, 256]

    gk_bf = _bf16(gk); lin_bf = _bf16(lin)

    in_maps = []
    for k in range(N_CORES):
        rows = device_rows[k]
        xr_core = np.zeros((NODES_PER_CORE, C * D), np.float32)
        valid = rows >= 0
        xr_core[valid] = xr[rows[valid]].reshape(-1, C * D)
        # channel-major x: xt[c, (tile, i, node)]
        xt = xr_core.reshape(TPC, TILE_N, C, D).transpose(2, 0, 3, 1)
        in_maps.append({
            'xt': _bf16(np.ascontiguousarray(xt.reshape(C, TPC * D * TILE_N))),
            'wc': wc_bf,
            'gk': gk_bf,
            'bias': bias,
            'lin': lin_bf,
        })
    return in_maps, device_rows, overflow_idx


# ----------------------------------------------------------------------------
# device kernel
# ----------------------------------------------------------------------------

def build_device(repeat=1, stages=5):
    import concourse.bacc as bacc
    import concourse.mybir as mybir
    from concourse.tile import TileContext

    f32, bf16 = mybir.dt.float32, mybir.dt.bfloat16
    AL = mybir.AluOpType

    nc = bacc.Bacc("TRN2", target_bir_lowering=False, debug=False,
                   num_devices=N_CORES)

    xt_d = nc.dram_tensor('xt', [C, TPC * D * TILE_N], bf16, kind='ExternalInput').ap()
    wc_d = nc.dram_tensor('wc', [C, S * NM55 * 36], bf16, kind='ExternalInput').ap()
    gk_d = nc.dram_tensor('gk', [C, S * 2 * C], bf16, kind='ExternalInput').ap()
    bias_d = nc.dram_tensor('bias', [C, S * 2], f32, kind='ExternalInput').ap()
    lin_d = nc.dram_tensor('lin', [C, 2 * C], bf16, kind='ExternalInput').ap()
    # transposed output: [c, (tile, z, node)]; host un-permutes and casts to f32
    out_d = nc.dram_tensor('out', [C, TPC * 4 * TILE_N], bf16, kind='ExternalOutput').ap()

    TD = D * TILE_N       # 1152: one tile's x block
    TV = NM55 * TILE_N    # 7040: one tile's V block

    with TileContext(nc) as tc:
        with (
            tc.tile_pool(name='const', bufs=1) as constp,
            tc.tile_pool(name='xt', bufs=1) as xtp,
            tc.tile_pool(name='vb', bufs=1) as vbp,
            tc.tile_pool(name='tg', bufs=1) as tgp,
            tc.tile_pool(name='ff', bufs=1) as ffp,
            tc.tile_pool(name='sb', bufs=1) as sbp,
            tc.tile_pool(name='facc', bufs=1) as faccp,
            tc.tile_pool(name='outt', bufs=1) as outp,
            tc.tile_pool(name='ps_misc', bufs=2, space='PSUM') as ps_m,
        ):
            wc_s = constp.tile([C, S * NM55 * 36], bf16)
            nc.sync.dma_start(out=wc_s[:], in_=wc_d[:])
            gk_s = constp.tile([C, S * 2 * C], bf16)
            nc.sync.dma_start(out=gk_s[:], in_=gk_d[:])
            bias_s = constp.tile([C, S * 2], f32)
            nc.sync.dma_start(out=bias_s[:], in_=bias_d[:])
            lin_s = constp.tile([C, 2 * C], bf16)
            nc.sync.dma_start(out=lin_s[:], in_=lin_d[:])

            for rep in range(repeat):
              for (t0, ng) in ((0, 4), (4, 4), (8, 2)):   # tile groups
                xt_t = xtp.tile([C, 4 * TD], bf16)
                nc.sync.dma_start(out=xt_t[:, 0:ng * TD],
                                  in_=xt_d[:, t0 * TD:(t0 + ng) * TD])

                for h in range(t0 // 2, (t0 + ng) // 2):  # pairs (t = 2h, 2h+1)
                    # ---- monomials for both tiles: vb2[c, (t2, m, n)] ----
                    vb2 = vbp.tile([C, 2 * TV], bf16)
                    xb2 = (xt_t[:, (2 * h - t0) * TD:(2 * h - t0 + 2) * TD]
                           .rearrange('p (t inn) -> p t inn', t=2))
                    vb2v = vb2[:, :].rearrange('p (t mn) -> p t mn', t=2)
                    for o in range(D):
                        nj = D - o
                        nc.vector.tensor_mul(
                            vb2v[:, :, OSTART[o] * TILE_N:(OSTART[o] + nj) * TILE_N],
                            xb2[:, :, 0:nj * TILE_N],
                            xb2[:, :, o * TILE_N:(o + nj) * TILE_N])
                    nc.vector.tensor_copy(
                        vb2v[:, :, 45 * TILE_N:54 * TILE_N], xb2)
                    nc.vector.memset(
                        vb2v[:, :, 54 * TILE_N:55 * TILE_N], 1.0)

                    if stages < 2:
                        ot = outp.tile([C, 2 * 4 * TILE_N], bf16, tag='ot')
                        nc.vector.tensor_copy(ot[:, 0:TILE_N], vb2[:, 0:TILE_N])
                        nc.vector.memset(ot[:, TILE_N:], 0.0)
                        nc.sync.dma_start(
                            out=out_d[:, 2 * h * 4 * TILE_N:(2 * h + 2) * 4 * TILE_N],
                            in_=ot[:])
                        continue

                    ot = outp.tile([C, 2 * 4 * TILE_N], bf16, tag='ot')
                    for t2 in range(2):
                        s = 2 * h + t2   # species == tile index
                        # ---- F[c,(z,n,i)] = sum_m V[c,(m,n)] wc[c,(s,m,zi)] ----
                        ff = ffp.tile([C, 4 * TILE_N * D], bf16)
                        v4 = (vb2[:, t2 * TV:(t2 + 1) * TV]
                              .rearrange('p (m n) -> p n m', m=NM55).unsqueeze(1))
                        wz = wc_s[:, s * NM55 * 36:(s + 1) * NM55 * 36].rearrange(
                            'p (m zi) -> p zi m', zi=36)
                        ff36 = ff[:, :].rearrange('p (zi n) -> p zi n', zi=36)
                        for zi0, zw in ((0, 8), (8, 8), (16, 8), (24, 8), (32, 4)):
                            tg = tgp.tile([C, 8 * TILE_N * NM55], bf16)
                            tg_v = tg[:, 0:zw * TILE_N * NM55].rearrange(
                                'p (zi n m) -> p zi n m', zi=zw, n=TILE_N)
                            nc.vector.tensor_mul(
                                tg_v,
                                v4.broadcast_to([C, zw, TILE_N, NM55]),
                                wz[:, zi0:zi0 + zw, :]
                                .unsqueeze(2).broadcast_to([C, zw, TILE_N, NM55]))
                            with nc.allow_low_precision(reason='fp32-internal reduce'):
                                nc.vector.tensor_reduce(
                                    ff36[:, zi0:zi0 + zw, :], tg_v,
                                    axis=mybir.AxisListType.X, op=AL.add)

                        # ---- f[c,(z,n)] = sum_i F[c,(z,n,i)] * x[c,(i,n)] ----
                        xv = (xt_t[:, (s - t0) * TD:(s - t0 + 1) * TD]
                              .rearrange('p (i n) -> p i n', i=D)
                              .unsqueeze(1).broadcast_to([C, 4, D, TILE_N]))
                        nc.vector.tensor_mul(
                            ff[:, :].rearrange('p (z i n) -> p z i n', z=4, i=D),
                            ff[:, :].rearrange('p (z i n) -> p z i n', z=4, i=D),
                            xv)
                        facc = faccp.tile([C, 4 * TILE_N], bf16)
                        with nc.allow_low_precision(reason='9-elem reduce'):
                            nc.vector.tensor_reduce(
                                facc[:, :].rearrange('p (z n) -> p z n', z=4),
                                ff[:, :].rearrange('p (z i n) -> p z n i', z=4, i=D),
                                axis=mybir.AxisListType.X, op=AL.add)

                        if stages < 5:
                            nc.vector.tensor_copy(
                                ot[:, t2 * 4 * TILE_N:(t2 + 1) * 4 * TILE_N], facc[:])
                            continue

                        # ---- gate matmuls: gate_half^T = gk_half^T @ f0^T ----
                        gps = ps_m.tile([C, 2 * TILE_N], f32, tag='misc')
                        nc.tensor.matmul(gps[:, 0:TILE_N],
                                         gk_s[:, s * 256:s * 256 + 128],
                                         facc[:, 0:TILE_N],
                                         start=True, stop=True)
                        nc.tensor.matmul(gps[:, TILE_N:2 * TILE_N],
                                         gk_s[:, s * 256 + 128:s * 256 + 256],
                                         facc[:, 0:TILE_N],
                                         start=True, stop=True)

                        # ---- fused bias + gating: fg = (gps + bias) * facc ----
                        fg = sbp.tile([C, 4 * TILE_N], bf16, tag='fg')
                        nc.vector.scalar_tensor_tensor(
                            out=fg[:, 0:TILE_N],
                            in0=gps[:, 0:TILE_N],
                            scalar=bias_s[:, 2 * s:2 * s + 1],
                            in1=facc[:, 0:TILE_N],
                            op0=AL.add, op1=AL.mult)
                        nc.vector.scalar_tensor_tensor(
                            out=fg[:, TILE_N:].rearrange('p (zz n) -> p zz n', zz=3),
                            in0=gps[:, TILE_N:2 * TILE_N].unsqueeze(1)
                            .broadcast_to([C, 3, TILE_N]),
                            scalar=bias_s[:, 2 * s + 1:2 * s + 2],
                            in1=facc[:, TILE_N:].rearrange('p (zz n) -> p zz n', zz=3),
                            op0=AL.add, op1=AL.mult)

                        # ---- linear (c-major): out^T [k, (z, n)] ----
                        ops_ = ps_m.tile([C, 4 * TILE_N], f32, tag='misc')
                        nc.tensor.matmul(ops_[:, 0:TILE_N], lin_s[:, 0:C],
                                         fg[:, 0:TILE_N], start=True, stop=True)
                        nc.tensor.matmul(ops_[:, TILE_N:4 * TILE_N], lin_s[:, C:2 * C],
                                         fg[:, TILE_N:4 * TILE_N], start=True, stop=True)
                        nc.vector.tensor_copy(
                            ot[:, t2 * 4 * TILE_N:(t2 + 1) * 4 * TILE_N], ops_[:])
                    nc.sync.dma_start(
                        out=out_d[:, 2 * h * 4 * TILE_N:(2 * h + 2) * 4 * TILE_N],
                        in_=ot[:])

    nc.compile()
    return nc


_NC_CACHE = {}


def _get_device(repeat=1, stages=5):
    key = (repeat, stages)
    if key not in _NC_CACHE:
        _NC_CACHE[key] = build_device(repeat, stages)
    return _NC_CACHE[key]


def kernel(**inputs):
    from concourse.bass_utils import run_bass_kernel_spmd

    in_maps, device_rows, overflow_idx = host_prepare(inputs)
    nc = _get_device(1)
    res = run_bass_kernel_spmd(nc, in_maps, list(range(N_CORES)))

    ntot = np.asarray(inputs['node_species']).shape[0]
    out = np.zeros((ntot, 4 * C), np.float32)
    for k in range(N_CORES):
        rows = device_rows[k]
        valid = rows >= 0
        # device output is [c, (tile, z, node)]; un-permute to [node, 512]
        a = np.asarray(res.results[k]['out'], np.float32).reshape(C, TPC, 4, TILE_N)
        o = np.empty((NODES_PER_CORE, 4 * C), np.float32)
        o[:, :C] = a[:, :, 0, :].transpose(1, 2, 0).reshape(NODES_PER_CORE, C)
        o[:, C:] = a[:, :, 1:4, :].transpose(1, 3, 0, 2).reshape(NODES_PER_CORE, 3 * C)
        out[rows[valid]] = o[valid]
    if len(overflow_idx):
        out[overflow_idx] = _numpy_forward(inputs, overflow_idx)
    return out


# revision 11
# speedup vs baseline: 1.0745x; 1.0745x over previous
"""Trainium2 Bass kernel for nn_EquivariantProductBasisBlock (MACE product-basis block).

Self-contained: host-side sharding/preprocessing + Bass/Tile device kernel on 8 cores.

Math (validated vs reference): per node n, channel c, species s, x = x[n,c,:] in R^9:
    out[z] = sum_i x_i * F[z,i],   F = C1 + C2 @ x + C3h @ y,   y = {x_j x_k}_{j<=k}
with C* the species/channel coefficient tables folded from (u*, w*) on the host.
Then gate = f0 @ gate_kernel[s] + gate_bias[s]; f0*=gate[:C]; f1*=gate[C:];
out = [f0 @ lin0, f1 @ lin1] / sqrt(C).

Device layout: channels on SBUF partitions, nodes species-sorted so every tile is
species-pure (tile t = species t, 10 tiles/core). The runtime bills ~40-50us per
instruction regardless of engine or size, so the program minimizes instruction
count: one bulk input DMA, monomial build batched over tile pairs (3D APs),
the per-tile V@W contraction in 8 broadcast-mul + 8 reduce ops (SBUF-bound
granularity), fused bias+gating via scalar_tensor_tensor, and per-pair output DMA.

Species overflow beyond 1024 nodes/species is computed on the host in numpy.
"""

import numpy as np

N_CORES = 8
C, D, S = 128, 9, 10
NM = 45           # deg-2 monomials
NROW = 56         # host slots: 45 y | 9 x | 1 one | 1 pad (overflow path)
NM55 = 55         # device slots per channel: 45 y | 9 x | 1 one
TILE_N = 128
TPC = S           # tiles per core (one per species)
NODES_PER_CORE = TPC * TILE_N          # 1280
CAP_PER_SPECIES = N_CORES * TILE_N     # 1024 device-handled nodes per species

# monomials ordered by diagonal offset o=k-j then j: slot(o,j) = OSTART[o]+j.
# Each V-build op is then pure step-1 (out/in0/in1 all contiguous runs).
OSTART = [0] * D
for o in range(1, D):
    OSTART[o] = OSTART[o - 1] + (D - (o - 1))
MONO_JK = [(j, j + o) for o in range(D) for j in range(D - o)]


# ----------------------------------------------------------------------------
# host math
# ----------------------------------------------------------------------------

def _build_xr(node_feats):
    n = node_feats.shape[0]
    x = np.empty((n, C, D), np.float32)
    x[:, :, 0] = node_feats[:, :C]
    x[:, :, 1:4] = node_feats[:, C:4 * C].reshape(n, C, 3)
    x[:, :, 4:9] = node_feats[:, 4 * C:].reshape(n, C, 5)
    return x


def _build_coeff_tables(i):
    def c3h(u3, w3):
        c3 = np.einsum('zijkp,spc->sczijk', u3, w3[:, :, :], optimize=True)
        out = np.zeros(c3.shape[:4] + (NM,), np.float64)
        for m, (j, k) in enumerate(MONO_JK):
            out[..., m] = c3[..., j, k] if j == k else c3[..., j, k] + c3[..., k, j]
        return out

    def c2(u2, w2):
        return np.einsum('zijp,spc->sczij', u2, w2, optimize=True)

    def c1(u1, w1):
        return np.einsum('zip,spc->sczi', u1, w1, optimize=True)

    h0 = c3h(i['u3_0e'], i['w3_0e']); h1 = c3h(i['u3_1o'], i['w3_1o'])
    q0 = c2(i['u2_0e'], i['w2_0e']);  q1 = c2(i['u2_1o'], i['w2_1o'])
    l0 = c1(i['u1_0e'], i['w1_0e']);  l1 = c1(i['u1_1o'], i['w1_1o'])

    W = np.zeros((S, C, NROW, 36), np.float64)
    W[:, :, 0:45, 0:9] = np.moveaxis(h0[:, :, 0], -1, -2)
    W[:, :, 45:54, 0:9] = np.moveaxis(q0[:, :, 0], -1, -2)
    W[:, :, 54, 0:9] = l0[:, :, 0]
    for z in range(3):
        sl = slice(9 + z * 9, 18 + z * 9)
        W[:, :, 0:45, sl] = np.moveaxis(h1[:, :, z], -1, -2)
        W[:, :, 45:54, sl] = np.moveaxis(q1[:, :, z], -1, -2)
        W[:, :, 54, sl] = l1[:, :, z]
    return W.astype(np.float32)   # [S, C, 56, 36]


def _numpy_forward(inputs, idx):
    """Reference-equivalent host computation for node subset idx (overflow path)."""
    i = {k: np.asarray(v) for k, v in inputs.items()}
    nf = i['node_feats'][idx]; sp = i['node_species'][idx]
    xr = _build_xr(nf)
    W = _build_coeff_tables(i)
    n = nf.shape[0]
    V = np.empty((n, C, NROW), np.float32)
    for m, (j, k) in enumerate(MONO_JK):
        V[:, :, m] = xr[:, :, j] * xr[:, :, k]
    V[:, :, 45:54] = xr
    V[:, :, 54] = 1.0
    V[:, :, 55] = 0.0
    F = np.einsum('ncm,ncmz->ncz', V, W[sp], optimize=True)
    f = np.einsum('nczi,nci->ncz', F.reshape(n, C, 4, D), xr, optimize=True)
    f0, f1 = f[:, :, 0], f[:, :, 1:4]
    gate = np.einsum('nc,nck->nk', f0, i['gate_kernel'][sp], optimize=True) + i['gate_bias'][sp]
    f0g = f0 * gate[:, :C]
    f1g = f1 * gate[:, C:, None]
    inv = 1.0 / np.sqrt(np.float32(C))
    o0 = np.einsum('nc,ck->nk', f0g, i['lin_w_0e'], optimize=True) * inv
    o1 = np.einsum('ncd,ck->nkd', f1g, i['lin_w_1o'], optimize=True) * inv
    return np.concatenate([o0.reshape(n, C), o1.reshape(n, C * 3)], axis=1).astype(np.float32)


def _bf16(x):
    import ml_dtypes
    return np.asarray(x, np.float32).astype(ml_dtypes.bfloat16)


def host_prepare(inputs):
    """Returns (per_core_inmaps, device_rows [N_CORES,1280] global node ids (-1 pad),
    overflow_idx)."""
    i = {k: np.asarray(v) for k, v in inputs.items()}
    sp = i['node_species']

    order = np.argsort(sp, kind='stable')
    sorted_sp = sp[order]
    device_rows = np.full((N_CORES, NODES_PER_CORE), -1, np.int64)
    overflow = []
    for s in range(S):
        ids = order[sorted_sp == s]
        dev = ids[:CAP_PER_SPECIES]
        overflow.append(ids[CAP_PER_SPECIES:])
        for k in range(N_CORES):
            chunk = dev[k * TILE_N:(k + 1) * TILE_N]
            device_rows[k, s * TILE_N: s * TILE_N + len(chunk)] = chunk
    overflow_idx = np.concatenate(overflow) if overflow else np.zeros(0, np.int64)

    xr = _build_xr(i['node_feats'])                       # [N, C, 9]
    W = _build_coeff_tables(i)                            # [S, C, 56, 36]
    # channel-major coefficient table: wc[c, (s, m55, zi36)]
    wc_bf = _bf16(np.ascontiguousarray(
        W[:, :, :55, :].transpose(1, 0, 2, 3).reshape(C, S * 55 * 36)))

    gk = np.zeros((C, S * 2 * C), np.float32)             # rows c, col s*256 + j
    for s in range(S):
        gk[:, s * 256:(s + 1) * 256] = i['gate_kernel'][s]

    bias = np.zeros((C, S * 2), np.float32)               # rows k2%128, col s*2 + half
    for s in range(S):
        bias[:, 2 * s] = i['gate_bias'][s, :C]
        bias[:, 2 * s + 1] = i['gate_bias'][s, C:]

    inv = 1.0 / np.sqrt(np.float32(C))
    lin = np.concatenate([i['lin_w_0e'] * inv, i['lin_w_1o'] * inv], axis=1)  # [128, 256]

    gk_bf = _bf16(gk); lin_bf = _bf16(lin)

    in_maps = []
    for k in range(N_CORES):
        rows = device_rows[k]
        xr_core = np.zeros((NODES_PER_CORE, C * D), np.float32)
        valid = rows >= 0
        xr_core[valid] = xr[rows[valid]].reshape(-1, C * D)
        # channel-major x: xt[c, (tile, i, node)]
        xt = xr_core.reshape(TPC, TILE_N, C, D).transpose(2, 0, 3, 1)
        in_maps.append({
            'xt': _bf16(np.ascontiguousarray(xt.reshape(C, TPC * D * TILE_N))),
            'wc': wc_bf,
            'gk': gk_bf,
            'bias': bias,
            'lin': lin_bf,
        })
    return in_maps, device_rows, overflow_idx


# ----------------------------------------------------------------------------
# device kernel
# ----------------------------------------------------------------------------

def build_device(repeat=1, stages=5):
    import concourse.bacc as bacc
    import concourse.mybir as mybir
    from concourse.tile import TileContext

    f32, bf16 = mybir.dt.float32, mybir.dt.bfloat16
    AL = mybir.AluOpType

    nc = bacc.Bacc("TRN2", target_bir_lowering=False, debug=False,
                   num_devices=N_CORES)

    xt_d = nc.dram_tensor('xt', [C, TPC * D * TILE_N], bf16, kind='ExternalInput').ap()
    wc_d = nc.dram_tensor('wc', [C, S * NM55 * 36], bf16, kind='ExternalInput').ap()
    gk_d = nc.dram_tensor('gk', [C, S * 2 * C], bf16, kind='ExternalInput').ap()
    bias_d = nc.dram_tensor('bias', [C, S * 2], f32, kind='ExternalInput').ap()
    lin_d = nc.dram_tensor('lin', [C, 2 * C], bf16, kind='ExternalInput').ap()
    # transposed output: [c, (tile, z, node)]; host un-permutes and casts to f32
    out_d = nc.dram_tensor('out', [C, TPC * 4 * TILE_N], bf16, kind='ExternalOutput').ap()

    TD = D * TILE_N       # 1152: one tile's x block
    TV = NM55 * TILE_N    # 7040: one tile's V block

    with TileContext(nc) as tc:
        with (
            tc.tile_pool(name='const', bufs=1) as constp,
            tc.tile_pool(name='xt', bufs=1) as xtp,
            tc.tile_pool(name='vb', bufs=1) as vbp,
            tc.tile_pool(name='tg', bufs=1) as tgp,
            tc.tile_pool(name='ff', bufs=1) as ffp,
            tc.tile_pool(name='sb', bufs=1) as sbp,
            tc.tile_pool(name='facc', bufs=1) as faccp,
            tc.tile_pool(name='outt', bufs=1) as outp,
            tc.tile_pool(name='ps_misc', bufs=2, space='PSUM') as ps_m,
        ):
            wc_s = constp.tile([C, S * NM55 * 36], bf16)
            nc.sync.dma_start(out=wc_s[:], in_=wc_d[:])
            gk_s = constp.tile([C, S * 2 * C], bf16)
            nc.sync.dma_start(out=gk_s[:], in_=gk_d[:])
            bias_s = constp.tile([C, S * 2], f32)
            nc.sync.dma_start(out=bias_s[:], in_=bias_d[:])
            lin_s = constp.tile([C, 2 * C], bf16)
            nc.sync.dma_start(out=lin_s[:], in_=lin_d[:])

            for rep in range(repeat):
              for (t0, ng) in ((0, 6), (6, 4)):   # tile groups
                xt_t = xtp.tile([C, 6 * TD], bf16)
                nc.sync.dma_start(out=xt_t[:, 0:ng * TD],
                                  in_=xt_d[:, t0 * TD:(t0 + ng) * TD])

                for h in range(t0 // 2, (t0 + ng) // 2):  # pairs (t = 2h, 2h+1)
                    # ---- monomials for both tiles: vb2[c, (t2, m, n)] ----
                    vb2 = vbp.tile([C, 2 * TV], bf16)
                    xb2 = (xt_t[:, (2 * h - t0) * TD:(2 * h - t0 + 2) * TD]
                           .rearrange('p (t inn) -> p t inn', t=2))
                    vb2v = vb2[:, :].rearrange('p (t mn) -> p t mn', t=2)
                    for o in range(D):
                        nj = D - o
                        nc.vector.tensor_mul(
                            vb2v[:, :, OSTART[o] * TILE_N:(OSTART[o] + nj) * TILE_N],
                            xb2[:, :, 0:nj * TILE_N],
                            xb2[:, :, o * TILE_N:(o + nj) * TILE_N])
                    nc.vector.tensor_copy(
                        vb2v[:, :, 45 * TILE_N:54 * TILE_N], xb2)
                    nc.vector.memset(
                        vb2v[:, :, 54 * TILE_N:55 * TILE_N], 1.0)

                    if stages < 2:
                        ot = outp.tile([C, 2 * 4 * TILE_N], bf16, tag='ot')
                        nc.vector.tensor_copy(ot[:, 0:TILE_N], vb2[:, 0:TILE_N])
                        nc.vector.memset(ot[:, TILE_N:], 0.0)
                        nc.sync.dma_start(
                            out=out_d[:, 2 * h * 4 * TILE_N:(2 * h + 2) * 4 * TILE_N],
                            in_=ot[:])
                        continue

                    ot = outp.tile([C, 2 * 4 * TILE_N], bf16, tag='ot')
                    for t2 in range(2):
                        s = 2 * h + t2   # species == tile index
                        # ---- F[c,(z,n,i)] = sum_m V[c,(m,n)] wc[c,(s,m,zi)] ----
                        ff = ffp.tile([C, 4 * TILE_N * D], f32)
                        v4 = (vb2[:, t2 * TV:(t2 + 1) * TV]
                              .rearrange('p (m n) -> p n m', m=NM55).unsqueeze(1))
                        wz = wc_s[:, s * NM55 * 36:(s + 1) * NM55 * 36].rearrange(
                            'p (m zi) -> p zi m', zi=36)
                        ff36 = ff[:, :].rearrange('p (zi n) -> p zi n', zi=36)
                        for zc in range(6):
                            zi0 = zc * 6
                            tg = tgp.tile([C, 6 * TILE_N * NM55], bf16)
                            tg_v = tg[:, :].rearrange(
                                'p (zi n m) -> p zi n m', zi=6, n=TILE_N)
                            nc.vector.tensor_mul(
                                tg_v,
                                v4.broadcast_to([C, 6, TILE_N, NM55]),
                                wz[:, zi0:zi0 + 6, :]
                                .unsqueeze(2).broadcast_to([C, 6, TILE_N, NM55]))
                            nc.vector.tensor_reduce(
                                ff36[:, zi0:zi0 + 6, :], tg_v,
                                axis=mybir.AxisListType.X, op=AL.add)

                        # ---- f[c,(z,n)] = sum_i F[c,(z,n,i)] * x[c,(i,n)] ----
                        gg = sbp.tile([C, 4 * TILE_N * D], bf16, tag='gg')
                        xv = (xt_t[:, (s - t0) * TD:(s - t0 + 1) * TD]
                              .rearrange('p (i n) -> p i n', i=D)
                              .unsqueeze(1).broadcast_to([C, 4, D, TILE_N]))
                        nc.vector.tensor_mul(
                            gg[:, :].rearrange('p (z i n) -> p z i n', z=4, i=D),
                            ff[:, :].rearrange('p (z i n) -> p z i n', z=4, i=D),
                            xv)
                        facc = faccp.tile([C, 4 * TILE_N], bf16)
                        with nc.allow_low_precision(reason='9-elem reduce'):
                            nc.vector.tensor_reduce(
                                facc[:, :].rearrange('p (z n) -> p z n', z=4),
                                gg[:, :].rearrange('p (z i n) -> p z n i', z=4, i=D),
                                axis=mybir.AxisListType.X, op=AL.add)

                        if stages < 5:
                            nc.vector.tensor_copy(
                                ot[:, t2 * 4 * TILE_N:(t2 + 1) * 4 * TILE_N], facc[:])
                            continue

                        # ---- gate matmuls: gate_half^T = gk_half^T @ f0^T ----
                        gps = ps_m.tile([C, 2 * TILE_N], f32, tag='misc')
                        nc.tensor.matmul(gps[:, 0:TILE_N],
                                         gk_s[:, s * 256:s * 256 + 128],
                                         facc[:, 0:TILE_N],
                                         start=True, stop=True)
                        nc.tensor.matmul(gps[:, TILE_N:2 * TILE_N],
                                         gk_s[:, s * 256 + 128:s * 256 + 256],
                                         facc[:, 0:TILE_N],
                                         start=True, stop=True)

                        # ---- fused bias + gating: fg = (gps + bias) * facc ----
                        fg = sbp.tile([C, 4 * TILE_N], bf16, tag='fg')
                        nc.vector.scalar_tensor_tensor(
                            out=fg[:, 0:TILE_N],
                            in0=gps[:, 0:TILE_N],
                            scalar=bias_s[:, 2 * s:2 * s + 1],
                            in1=facc[:, 0:TILE_N],
                            op0=AL.add, op1=AL.mult)
                        nc.vector.scalar_tensor_tensor(
                            out=fg[:, TILE_N:].rearrange('p (zz n) -> p zz n', zz=3),
                            in0=gps[:, TILE_N:2 * TILE_N].unsqueeze(1)
                            .broadcast_to([C, 3, TILE_N]),
                            scalar=bias_s[:, 2 * s + 1:2 * s + 2],
                            in1=facc[:, TILE_N:].rearrange('p (zz n) -> p zz n', zz=3),
                            op0=AL.add, op1=AL.mult)

                        # ---- linear (c-major): out^T [k, (z, n)] ----
                        ops_ = ps_m.tile([C, 4 * TILE_N], f32, tag='misc')
                        nc.tensor.matmul(ops_[:, 0:TILE_N], lin_s[:, 0:C],
                                         fg[:, 0:TILE_N], start=True, stop=True)
                        nc.tensor.matmul(ops_[:, TILE_N:4 * TILE_N], lin_s[:, C:2 * C],
                                         fg[:, TILE_N:4 * TILE_N], start=True, stop=True)
                        nc.vector.tensor_copy(
                            ot[:, t2 * 4 * TILE_N:(t2 + 1) * 4 * TILE_N], ops_[:])
                    nc.sync.dma_start(
                        out=out_d[:, 2 * h * 4 * TILE_N:(2 * h + 2) * 4 * TILE_N],
                        in_=ot[:])

    nc.compile()
    return nc


_NC_CACHE = {}


def _get_device(repeat=1, stages=5):
    key = (repeat, stages)
    if key not in _NC_CACHE:
        _NC_CACHE[key] = build_device(repeat, stages)
    return _NC_CACHE[key]


def kernel(**inputs):
    from concourse.bass_utils import run_bass_kernel_spmd

    in_maps, device_rows, overflow_idx = host_prepare(inputs)
    nc = _get_device(1)
    res = run_bass_kernel_spmd(nc, in_maps, list(range(N_CORES)))

    ntot = np.asarray(inputs['node_species']).shape[0]
    out = np.zeros((ntot, 4 * C), np.float32)
    for k in range(N_CORES):
        rows = device_rows[k]
        valid = rows >= 0
        # device output is [c, (tile, z, node)]; un-permute to [node, 512]
        a = np.asarray(res.results[k]['out'], np.float32).reshape(C, TPC, 4, TILE_N)
        o = np.empty((NODES_PER_CORE, 4 * C), np.float32)
        o[:, :C] = a[:, :, 0, :].transpose(1, 2, 0).reshape(NODES_PER_CORE, C)
        o[:, C:] = a[:, :, 1:4, :].transpose(1, 3, 0, 2).reshape(NODES_PER_CORE, 3 * C)
        out[rows[valid]] = o[valid]
    if len(overflow_idx):
        out[overflow_idx] = _numpy_forward(inputs, overflow_idx)
    return out
